# revision 1
# baseline (speedup 1.0000x reference)
"""CARAFE content-aware upsampling kernel for 8 Trainium2 NeuronCores.

Math: out[b,c,2h+p,2w+q] = sum_{ki,kj} x[b,c,h+ki-2,w+kj-2] * kappa[b,ki*5+kj,2h+p,2w+q]

Mapping: per low-res row h and tap-row ki this is a banded matmul over the
width window w' against the (transposed) x row:

    out[(wl,p,q), c] += sum_{v} Band_{h,ki,w0}[v, (wl,p,q)] * xT[h+ki-2][w0*32+v, c]

so the device kernel is a dense stream of PSUM-accumulating float32r matmuls
(K=36, M=128, N=256), 5 per output tile.

Band matrices are mostly structural zeros (5-wide staircase band in a 36x128
window). The first two row-groups ship fully dense (writing the zeros into the
two rotating SBUF band buffers); later groups ship only the 20-element
diagonal runs and DMA-scatter them in place with a hand-built flat
(partition-crossing) access pattern — the zeros persist across buffer reuse
because the sparsity structure is identical for every group.

Sharding: 8 cores = batch (4) x low-res-row halves (2).
"""

import sys

import numpy as np

if "/opt/trn_rl_repo" not in sys.path:
    sys.path.insert(0, "/opt/trn_rl_repo")

B, C, H, W = 4, 256, 64, 64
K, R = 5, 2           # kernel_size, ratio
PAD = K // 2
NCORES = 8
HL = H // 2           # low-res rows per core
VW = 32 + 2 * PAD     # matmul contraction window (w' partitions)
HROWS = HL + 2 * PAD  # x rows staged per core
WPAD = W + 2 * PAD
HB = 4                # low-res rows per band/output group
NG = HL // HB         # 8 groups
NDENSE = 2            # groups shipped dense
NBUF = 2              # rotating band buffers
MW = 160              # padded band width per (ki,w0): real cols at [16, 144)
RUN = (K - 1) * R * R + R * R  # 20: diagonal run length per partition

_cache = {}


def _build(**opts):
    key = tuple(sorted(opts.items())) or "nc"
    if key in _cache:
        return _cache[key]
    import bass_rust
    import concourse.tile as tile
    from concourse import bacc, mybir

    f32 = mybir.dt.float32
    mm_dt = mybir.dt.float32r
    skip_mm = opts.get("skip_mm", False)
    skip_out = opts.get("skip_out", False)
    warm = opts.get("warm", 0)
    b0split = opts.get("b0split", True)
    outsplit = opts.get("outsplit", True)

    nc = bacc.Bacc(
        "TRN2", target_bir_lowering=False, debug=False, num_devices=NCORES
    )
    xs_d = nc.dram_tensor("xs", [HROWS, WPAD, C], mm_dt, kind="ExternalInput")
    bd_d = nc.dram_tensor(
        "bdense", [NDENSE, VW, HB, 10, MW], mm_dt, kind="ExternalInput"
    )
    bs_d = nc.dram_tensor(
        "bsparse", [NG - NDENSE, VW, HB, 10, RUN], mm_dt, kind="ExternalInput"
    )
    o_d = nc.dram_tensor("out", [HL, 2, 128, C], f32, kind="ExternalOutput")

    XSP = 8  # x rows in the early slab (covers the first band group)
    with tile.TileContext(nc) as tc:
        with (
            tc.tile_pool(name="xp", bufs=1) as xp,
            tc.tile_pool(name="bp", bufs=1) as bp,
            tc.tile_pool(name="pp", bufs=4, space="PSUM") as pp,
            tc.tile_pool(name="op", bufs=3) as op,
        ):
            # First band buffer before x so PE's first dependency chain is
            # short; x staged as [w' partition, h row, c] per width half so
            # every matmul rhs is a zero-copy view with base partition 0.
            bts = [
                bp.tile([VW, HB, 10, MW], mm_dt, tag=f"bt{i}", name=f"bt{i}")
                for i in range(NBUF)
            ]
            if warm:
                # Dummy matmuls keep PE busy through the input-DMA head so
                # the p-state ramp completes before the first real matmul.
                wt = xp.tile([32, 48], mm_dt, tag="warm")
                nc.vector.memset(wt[:], 0.0)
                wps = pp.tile([32, 16], f32, tag="warmps")
                for _ in range(warm):
                    nc.tensor.matmul(wps[:], wt[:, 0:32], wt[:, 32:48])
            if b0split:
                nc.scalar.dma_start(bts[0][:, 0, :, :], bd_d.ap()[0, :, 0])
            else:
                nc.scalar.dma_start(bts[0][:], bd_d.ap()[0])
            if NBUF > NDENSE:
                nc.gpsimd.memset(bts[NBUF - 1][:].bitcast(f32), 0.0)
            xa, xb = [], []
            for w0 in range(2):
                src = xs_d.ap()[:, 32 * w0 : 32 * w0 + VW, :].transpose((1, 0, 2))
                t = xp.tile([VW, XSP, C], mm_dt, tag=f"xa{w0}")
                nc.scalar.dma_start(t[:], src[:, :XSP, :])
                xa.append(t)
            if b0split:
                nc.scalar.dma_start(bts[0][:, 1:, :, :], bd_d.ap()[0, :, 1:])
            for w0 in range(2):
                src = xs_d.ap()[:, 32 * w0 : 32 * w0 + VW, :].transpose((1, 0, 2))
                t = xp.tile([VW, HROWS - XSP, C], mm_dt, tag=f"xb{w0}")
                nc.scalar.dma_start(t[:], src[:, XSP:, :])
                xb.append(t)

            def xrow(w0, r):
                return xa[w0][:, r, :] if r < XSP else xb[w0][:, r - XSP, :]

            for hb in range(NG):
                bt = bts[hb % NBUF]
                if 0 < hb < NDENSE:
                    nc.scalar.dma_start(bt[:], bd_d.ap()[hb])
                elif hb >= NDENSE:
                    # Sparse refresh: overwrite only the diagonal runs.
                    # Flat AP: per-partition pitch is HB*10*MW elements, so
                    # step pitch+R*R walks the band staircase (v, 4v..4v+20).
                    pitch = HB * 10 * MW
                    dst = bt[:]
                    dst.ap = bass_rust.VecI64Pair(
                        [(pitch + R * R, VW), (10 * MW, HB), (MW, 10), (1, RUN)]
                    )
                    nc.scalar.dma_start(dst, bs_d.ap()[hb - NDENSE])
                ot = op.tile([128, HB, 2, C], f32)
                for hh in range(HB):
                    h = hb * HB + hh
                    for w0 in range(2):
                        ps = pp.tile([128, C], f32)
                        if not skip_mm:
                            for ki in range(K):
                                lhsT = bt[:, hh, ki * 2 + w0, 16:144]
                                nc.tensor.matmul(
                                    ps[:],
                                    lhsT,
                                    xrow(w0, h + ki),
                                    start=(ki == 0),
                                    stop=(ki == K - 1),
                                )
                        else:
                            nc.gpsimd.memset(ps[:], 0.0)
                        nc.vector.tensor_copy(ot[:, hh, w0, :], ps[:])
                if not skip_out:
                    if outsplit:
                        # Final group ships per-row so the kernel tail is one
                        # quarter-size transfer.
                        step = 1 if hb == NG - 1 else 2
                        for part in range(HB // step):
                            r0 = hb * HB + step * part
                            nc.sync.dma_start(
                                o_d.ap()[r0 : r0 + step].transpose((2, 0, 1, 3)),
                                ot[:, step * part : step * (part + 1)],
                            )
                    else:
                        nc.sync.dma_start(
                            o_d.ap()[hb * HB : (hb + 1) * HB].transpose((2, 0, 1, 3)),
                            ot[:],
                        )

    nc.compile()
    _cache[key] = nc
    return nc


def _prep_core(x_pad_t, kern, core):
    """Per-core inputs: staged x slab + dense/sparse band payloads."""
    b, hh = divmod(core, 2)
    h0 = hh * HL
    xs = np.ascontiguousarray(x_pad_t[b, h0 : h0 + HROWS])  # [36, 68, C]

    # kern[b]: [25, 128, 128] -> [ki, kj, h, p, w0, wl, q] slice for this core
    ks = kern[b].reshape(K, K, H, R, W, R)[:, :, h0 : h0 + HL]
    ks = ks.reshape(K, K, HL, R, 2, 32, R)

    # Sparse runs: S[h, v, (ki,w0), o, (p,q)] = kappa at kj=4-o, wl=v-4+o.
    t2 = np.transpose(ks, (2, 0, 4, 1, 5, 3, 6))  # [h, ki, w0, kj, wl, p, q]
    t2 = np.ascontiguousarray(t2).reshape(HL, 10, K, 32, R * R)
    s = np.zeros((HL, VW, 10, K, R * R), np.float32)
    for o in range(K):
        kj = K - 1 - o
        s[:, kj : kj + 32, :, o, :] = np.transpose(t2[:, :, kj], (0, 2, 1, 3))
    s = s.reshape(HL, VW, 10, RUN)

    # Dense payload for the first NDENSE groups: runs scattered on the
    # diagonal (v, [4v, 4v+20)) of the padded MW-wide band, zeros elsewhere.
    d = np.zeros((NDENSE * HB, VW, 10, MW), np.float32)
    for v in range(VW):
        d[:, v, :, R * R * v : R * R * v + RUN] = s[: NDENSE * HB, v]
    bdense = np.ascontiguousarray(
        np.transpose(d.reshape(NDENSE, HB, VW, 10, MW), (0, 2, 1, 3, 4))
    )
    bsparse = np.ascontiguousarray(
        np.transpose(s[NDENSE * HB :].reshape(NG - NDENSE, HB, VW, 10, RUN),
                     (0, 2, 1, 3, 4))
    )
    return {"xs": xs, "bdense": bdense, "bsparse": bsparse}


def _assemble(results):
    out = np.empty((B, C, H * R, W * R), np.float32)
    for i in range(NCORES):
        b, hh = divmod(i, 2)
        h0 = hh * HL
        o = results[i]["out"].reshape(HL, 2, 32, R, R, C)  # [h,w0,wl,p,q,c]
        oc = np.transpose(o, (5, 0, 3, 1, 2, 4)).reshape(C, HL * R, W * R)
        out[b, :, h0 * R : (h0 + HL) * R, :] = oc
    return out


def _in_maps(x, kern):
    x_pad_t = np.pad(
        np.transpose(np.asarray(x, np.float32), (0, 2, 3, 1)),
        ((0, 0), (PAD, PAD), (PAD, PAD), (0, 0)),
    )
    kern = np.asarray(kern, np.float32)
    return [_prep_core(x_pad_t, kern, i) for i in range(NCORES)]


def kernel(x, kernel, kernel_size, ratio):
    assert int(kernel_size) == K and int(ratio) == R
    x = np.asarray(x)
    assert x.shape == (B, C, H, W), x.shape
    nc = _build()
    from concourse.bass_utils import run_bass_kernel_spmd

    res = run_bass_kernel_spmd(nc, _in_maps(x, kernel), core_ids=list(range(NCORES)))
    return _assemble(res.results)



# revision 16
# speedup vs baseline: 1.1758x; 1.1758x over previous
"""CARAFE content-aware upsampling kernel for 8 Trainium2 NeuronCores.

Math: out[b,c,2h+p,2w+q] = sum_{ki,kj} x[b,c,h+ki-2,w+kj-2] * kappa[b,ki*5+kj,2h+p,2w+q]

Per low-res row h and width-half w0, the 128 outputs (wl,p,q) form a banded
matmul over the 36-wide w' window against x rows. Tap rows are fused in pairs
into the contraction dim via shift-staged x: the x tile holds two partition
blocks kb=0,1 where block kb row j = x[j+kb], so

    mm(kseg):  psum += Band_kseg[72, 128]^T @ xt[0:72, h + 2*kseg, :]

covers taps (2*kseg, 2*kseg+1), and a final K=36 matmul covers tap 4 —
3 matmuls per output tile instead of 5.

Everything streams bf16 (fp32 PSUM accumulate); output is written fp16 and
widened on host. Band matrices are structural-zero staircases; zeros are
established once (dense group-0 DMA for buffer A, memsets for buffer B) and
later groups scatter only the 40-element fused runs (both w0 halves
interleaved) with flat partition-crossing APs.

Sharding: 8 cores = batch (4) x low-res-row halves (2).
"""

import sys

import numpy as np

if "/opt/trn_rl_repo" not in sys.path:
    sys.path.insert(0, "/opt/trn_rl_repo")

B, C, H, W = 4, 256, 64, 64
K, R = 5, 2           # kernel_size, ratio
PAD = K // 2
NCORES = 8
HL = H // 2           # low-res rows per core
VW = 32 + PAD + PAD   # contraction window width (36)
NKB = 2               # stacked tap rows per paired matmul
KP = NKB * VW         # 72 partitions for paired matmuls
XROWS = VW            # x rows staged per block (36)
HB = 8                # low-res rows per band/output group
NG = HL // HB         # 4 groups
RUN = K * 8           # 40: (kj 5) x (w0 2) x (p 2) x (q 2) fused run
MWL = VW + R * R      # 40 wl' positions per segment
SEG = MWL * 8         # 320 elems per (kseg,hh) segment
XH = 8                # x head rows (pipelining split)
PADB = 32             # scratch elems at the head of each BT partition
PITCH = PADB + 2 * HB * SEG  # BT per-partition pitch (elements)
WA = 256              # kb1 base-32 windowed-dense column width
WB = 96               # kb1 base-64 windowed-dense column width

_cache = {}


def _build(**opts):
    key = tuple(sorted(opts.items())) or "nc"
    if key in _cache:
        return _cache[key]
    import bass_rust
    import concourse.tile as tile
    from concourse import bacc, mybir

    f32 = mybir.dt.float32
    f16 = mybir.dt.float16
    bf16 = mybir.dt.bfloat16

    nc = bacc.Bacc(
        "TRN2", target_bir_lowering=False, debug=False, num_devices=NCORES
    )
    xs_d = nc.dram_tensor("xs", [2, KP, XROWS, C], bf16, kind="ExternalInput")
    bd0_d = nc.dram_tensor("bd0", [KP, 2, HB, SEG], bf16, kind="ExternalInput")
    b40_d = nc.dram_tensor("b40", [VW, HB, SEG], bf16, kind="ExternalInput")
    bts_d = nc.dram_tensor(
        "bts", [NG - 1, VW, 2 * HB, RUN], bf16, kind="ExternalInput"
    )
    b4s_d = nc.dram_tensor(
        "b4s", [NG - 1, VW, HB, RUN], bf16, kind="ExternalInput"
    )
    btw_d = nc.dram_tensor(
        "btw", [NG - 1, 32, 2 * HB, WA], bf16, kind="ExternalInput"
    )
    btd_d = nc.dram_tensor(
        "btd", [NG - 1, 8, 2 * HB, WB], bf16, kind="ExternalInput"
    )
    o_d = nc.dram_tensor("out", [HL, 2, 128, C], f16, kind="ExternalOutput")

    with tile.TileContext(nc) as tc:
        with (
            tc.tile_pool(name="xp", bufs=1) as xp,
            tc.tile_pool(name="bp", bufs=1) as bp,
            tc.tile_pool(name="pp", bufs=4, space="PSUM") as pp,
            tc.tile_pool(name="op", bufs=3) as op,
        ):
            # Rotating band buffers. BT is flat [KP, PADB + 16*SEG]: a 32-elem
            # scratch head per partition absorbs the base-32 scatter's dummy
            # rows (DMA partition bases must be 0/32/64), then 16 (kseg, hh)
            # segments of 320 in (wl', p, q, w0) element order.
            bts = [
                bp.tile([KP, PADB + 2 * HB * SEG], bf16, tag=f"bt{i}", name=f"bt{i}")
                for i in range(2)
            ]
            b4s = [
                bp.tile([VW, HB, MWL, 2, 2, 2], bf16, tag=f"b4{i}", name=f"b4{i}")
                for i in range(2)
            ]

            def bt_dense(bt, lo, hi):
                dst = bt[:]
                dst.ap = bass_rust.VecI64Pair(
                    [(PITCH, KP), (HB * SEG, 2), (1, (hi - lo) * SEG)]
                )
                dst.offset = dst.offset + PADB + lo * SEG
                return dst

            # Buffer A: dense group 0 (zeros ride along with the data), split
            # so the first rows' bands land before the rest.
            XD = 2  # head rows of the dense group-0 transfer
            nc.scalar.dma_start(bt_dense(bts[0], 0, XD), bd0_d.ap()[:, :, :XD])
            nc.scalar.dma_start(b4s[0][:, :XD], b40_d.ap()[:, :XD])
            nc.scalar.dma_start(bt_dense(bts[0], XD, HB), bd0_d.ap()[:, :, XD:])
            nc.scalar.dma_start(b4s[0][:, XD:], b40_d.ap()[:, XD:])
            # Buffer B zeros: engine memsets, overlapped with group-0 compute.
            nc.gpsimd.memset(bts[1][:].bitcast(f32), 0.0)
            nc.vector.memset(b4s[1][:].bitcast(f32), 0.0)

            # x: head rows first so the first matmuls' inputs land early.
            xa, xb = [], []
            for w0 in range(2):
                t = xp.tile([KP, XH, C], bf16, tag=f"xa{w0}")
                nc.sync.dma_start(t[:], xs_d.ap()[w0, :, :XH])
                xa.append(t)
            for w0 in range(2):
                t = xp.tile([KP, XROWS - XH, C], bf16, tag=f"xb{w0}")
                nc.sync.dma_start(t[:], xs_d.ap()[w0, :, XH:])
                xb.append(t)

            def xrow(w0, r, parts):
                t = xa[w0] if r < XH else xb[w0]
                return t[0:parts, r if r < XH else r - XH, :]

            pitch4 = HB * SEG

            def bt_lhsT(bt, kseg, hh, w0):
                ap = bt[:]
                ap.ap = bass_rust.VecI64Pair([(PITCH, KP), (2, 128)])
                ap.offset = (
                    ap.offset
                    + PADB
                    + (kseg * HB + hh) * SEG
                    + 8 * (K - 1)
                    + w0
                )
                return ap

            def bt_scatter(bt, p0, nrows, voff):
                # Staircase scatter into partitions [p0, p0+nrows): partition
                # p0+i holds v = p0+i-VW*kb... encoded via voff = 8*v(i=0).
                dst = bt[:]
                dst.ap = bass_rust.VecI64Pair(
                    [(PITCH + 8, nrows), (SEG, 2 * HB), (1, RUN)]
                )
                dst.offset = dst.offset + p0 * PITCH + PADB + voff
                return dst

            for g in range(NG):
                bt, b4 = bts[g % 2], b4s[g % 2]
                if g >= 1:
                    # Sparse refresh: overwrite only the fused runs. The kb=1
                    # block needs partition bases 32/64; the base-32 piece
                    # leads with 4 zero rows that land in the scratch heads.
                    nc.scalar.dma_start(
                        bt_scatter(bt, 0, VW, 0), bts_d.ap()[g - 1]
                    )
                    # kb1 (partitions 36..72): staircase scatters are illegal
                    # off partition 0, so ship windowed-dense rectangles from
                    # the aligned bases 32/64. The base-32 piece's 4 leading
                    # rows only overwrite structural zeros with zeros.
                    dstw = bt[:]
                    dstw.ap = bass_rust.VecI64Pair(
                        [(PITCH, 32), (SEG, 2 * HB), (1, WA)]
                    )
                    dstw.offset = dstw.offset + 32 * PITCH + PADB
                    nc.scalar.dma_start(dstw, btw_d.ap()[g - 1])
                    dstd = bt[:]
                    dstd.ap = bass_rust.VecI64Pair(
                        [(PITCH, 8), (SEG, 2 * HB), (1, WB)]
                    )
                    dstd.offset = dstd.offset + 64 * PITCH + PADB + 8 * 28
                    nc.scalar.dma_start(dstd, btd_d.ap()[g - 1])
                    dst4 = b4[:]
                    dst4.ap = bass_rust.VecI64Pair(
                        [(pitch4 + 8, VW), (SEG, HB), (1, RUN)]
                    )
                    nc.scalar.dma_start(dst4, b4s_d.ap()[g - 1])
                ot = op.tile([128, HB, 2, C], f16)
                for hh in range(HB):
                    h = g * HB + hh
                    ps = pp.tile([128, 2, C], f32)
                    for w0 in range(2):
                        for kseg in range(2):
                            nc.tensor.matmul(
                                ps[:, w0, :],
                                bt_lhsT(bt, kseg, hh, w0),
                                xrow(w0, h + 2 * kseg, KP),
                                start=(kseg == 0),
                                stop=False,
                            )
                        nc.tensor.matmul(
                            ps[:, w0, :],
                            b4[:, hh, K - 1 : K - 1 + 32, :, :, w0],
                            xrow(w0, h + 4, VW),
                            start=False,
                            stop=True,
                        )
                    if h % 2 == 0:
                        nc.vector.tensor_copy(ot[:, hh, :, :], ps[:])
                    else:
                        nc.scalar.copy(ot[:, hh, :, :], ps[:])
                # Final group ships in shrinking chunks so the tail is short.
                chunks = [(0, HB)] if g < NG - 1 else [(0, 4), (4, 6), (6, 7), (7, 8)]
                for lo, hi in chunks:
                    r0 = g * HB
                    nc.sync.dma_start(
                        o_d.ap()[r0 + lo : r0 + hi].transpose((2, 0, 1, 3)),
                        ot[:, lo:hi],
                    )

    nc.compile()
    _cache[key] = nc
    return nc


def _band_runs(kern_b, h0):
    """S[ki, v, h, 40]: fused scatter runs for this core's 32 rows.

    Run element j = dwl*8 + p*4 + q*2 + w0 holds kappa for tap (ki, kj=4-dwl)
    at output (h, wl=v-kj, w0, p, q); zero where wl falls outside [0, 32).
    """
    ks = kern_b.reshape(K, K, H, R, W, R)[:, :, h0 : h0 + HL]
    V = ks.reshape(K, K, HL, R, 2, 32, R)  # [ki, kj, h, p, w0, wl, q]
    S = np.zeros((K, VW, HL, K, R, R, 2), np.float32)
    for dwl in range(K):
        kj = K - 1 - dwl
        # V[:, kj] dims [ki, h, p, w0, wl, q] -> [ki, wl, h, p, q, w0]
        S[:, kj : kj + 32, :, dwl] = np.transpose(V[:, kj], (0, 4, 1, 2, 5, 3))
    return S.reshape(K, VW, HL, RUN)


def _prep_core(xpad_t, kern, core, bf16):
    b, hh = divmod(core, 2)
    h0 = hh * HL

    # x blocks: block kb row j = padded x row h0+j+kb (zero past the end).
    base = xpad_t[b]  # [H+4, W+4, C]
    ext = np.concatenate([base, np.zeros((1,) + base.shape[1:], base.dtype)], 0)
    xs = np.empty((2, KP, XROWS, C), np.float32)
    for w0 in range(2):
        for kb in range(NKB):
            blk = ext[h0 + kb : h0 + kb + XROWS, 32 * w0 : 32 * w0 + VW]
            xs[w0, kb * VW : (kb + 1) * VW] = np.transpose(blk, (1, 0, 2))

    S = _band_runs(kern[b], h0)  # [ki, v, h, RUN]
    Sg = S.reshape(K, VW, NG, HB, RUN)

    # Sparse payloads for groups 1..NG-1: [g, kb, v, (kseg, hh), RUN].
    bts = np.empty((NG, NKB, VW, 2, HB, RUN), np.float32)
    for kseg in range(2):
        for kb in range(NKB):
            bts[:, kb, :, kseg] = np.transpose(Sg[2 * kseg + kb], (1, 0, 2, 3))
    b4s = np.transpose(Sg[K - 1], (1, 0, 2, 3))  # [g, v, hh, RUN]

    # Dense group 0: runs scattered on the 8v staircase, zeros elsewhere.
    d0 = np.zeros((NKB, VW, 2, HB, SEG), np.float32)
    d40 = np.zeros((VW, HB, SEG), np.float32)
    for v in range(VW):
        d0[:, v, :, :, 8 * v : 8 * v + RUN] = bts[0, :, v]
        d40[v, :, 8 * v : 8 * v + RUN] = b4s[0, v]
    # device layout [KP(kb,v), kseg, hh, SEG]
    d0 = d0.reshape(KP, 2, HB, SEG)
    d40 = d40.reshape(VW, HB, SEG)

    # kb0 scatters sparse runs from partition 0; kb1 ships as two
    # windowed-dense rectangles (zeros included) from bases 32 and 64.
    a = bts[1:].reshape(NG - 1, NKB, VW, 2 * HB, RUN)
    ssrc = np.ascontiguousarray(a[:, 0])
    btw = np.zeros((NG - 1, 32, 2 * HB, WA), np.float32)
    for v in range(28):
        btw[:, 4 + v, :, 8 * v : 8 * v + RUN] = a[:, 1, v]
    btd = np.zeros((NG - 1, 8, 2 * HB, WB), np.float32)
    for j in range(8):
        btd[:, j, :, 8 * j : 8 * j + RUN] = a[:, 1, 28 + j]

    return {
        "xs": xs.astype(bf16),
        "bd0": d0.astype(bf16),
        "b40": d40.astype(bf16),
        "bts": ssrc.astype(bf16),
        "b4s": b4s[1:].astype(bf16),
        "btw": btw.astype(bf16),
        "btd": btd.astype(bf16),
    }


def _assemble(results):
    out = np.empty((B, C, H * R, W * R), np.float32)
    for i in range(NCORES):
        b, hh = divmod(i, 2)
        h0 = hh * HL
        o = np.asarray(results[i]["out"], np.float32)
        o = o.reshape(HL, 2, 32, R, R, C)  # [h, w0, wl, p, q, c]
        oc = np.transpose(o, (5, 0, 3, 1, 2, 4)).reshape(C, HL * R, W * R)
        out[b, :, h0 * R : (h0 + HL) * R, :] = oc
    return out


def _in_maps(x, kern):
    import ml_dtypes

    bf16 = ml_dtypes.bfloat16
    x_pad_t = np.pad(
        np.transpose(np.asarray(x, np.float32), (0, 2, 3, 1)),
        ((0, 0), (PAD, PAD), (PAD, PAD), (0, 0)),
    )
    kern = np.asarray(kern, np.float32)
    return [_prep_core(x_pad_t, kern, i, bf16) for i in range(NCORES)]


def kernel(x, kernel, kernel_size, ratio):
    assert int(kernel_size) == K and int(ratio) == R
    x = np.asarray(x)
    assert x.shape == (B, C, H, W), x.shape
    nc = _build()
    from concourse.bass_utils import run_bass_kernel_spmd

    res = run_bass_kernel_spmd(nc, _in_maps(x, kernel), core_ids=list(range(NCORES)))
    return _assemble(res.results)


# revision 19
# speedup vs baseline: 1.2368x; 1.0519x over previous
"""CARAFE content-aware upsampling kernel for 8 Trainium2 NeuronCores.

Math: out[b,c,2h+p,2w+q] = sum_{ki,kj} x[b,c,h+ki-2,w+kj-2] * kappa[b,ki*5+kj,2h+p,2w+q]

Per low-res row h and width-half w0, the 128 outputs (wl,p,q) form a banded
matmul over the 36-wide w' window against x rows. Tap rows are fused in pairs
into the contraction dim via shift-staged x: the x tile holds two partition
blocks kb=0,1 where block kb row j = x[j+kb], so

    mm(kseg):  psum += Band_kseg[72, 128]^T @ xt[0:72, h + 2*kseg, :]

covers taps (2*kseg, 2*kseg+1), and a final K=36 matmul covers tap 4 —
3 matmuls per output tile instead of 5.

Everything streams bf16 (fp32 PSUM accumulate); output is written fp16 and
widened on host. Band matrices are structural-zero staircases; zeros are
established once (dense group-0 DMA for buffer A, memsets for buffer B) and
later groups scatter only the 40-element fused runs (both w0 halves
interleaved) with flat partition-crossing APs.

Sharding: 8 cores = batch (4) x low-res-row halves (2).
"""

import sys

import numpy as np

if "/opt/trn_rl_repo" not in sys.path:
    sys.path.insert(0, "/opt/trn_rl_repo")

B, C, H, W = 4, 256, 64, 64
K, R = 5, 2           # kernel_size, ratio
PAD = K // 2
NCORES = 8
HL = H // 2           # low-res rows per core
VW = 32 + PAD + PAD   # contraction window width (36)
NKB = 2               # stacked tap rows per paired matmul
KP = NKB * VW         # 72 partitions for paired matmuls
XROWS = VW            # x rows staged per block (36)
HB = 8                # low-res rows per band/output group
NG = HL // HB         # 4 groups
RUN = K * 8           # 40: (kj 5) x (w0 2) x (p 2) x (q 2) fused run
MWL = VW + R * R      # 40 wl' positions per segment
SEG = MWL * 8         # 320 elems per (kseg,hh) segment
XH = 8                # x head rows (pipelining split)
PADB = 32             # scratch elems at the head of each BT partition
PITCH = PADB + 2 * HB * SEG  # BT per-partition pitch (elements)
WA = 256              # kb1 base-32 windowed-dense column width
WB = 96               # kb1 base-64 windowed-dense column width

_cache = {}


def _build(**opts):
    key = tuple(sorted(opts.items())) or "nc"
    if key in _cache:
        return _cache[key]
    import bass_rust
    import concourse.tile as tile
    from concourse import bacc, mybir

    f32 = mybir.dt.float32
    f16 = mybir.dt.float16
    bf16 = mybir.dt.bfloat16

    nc = bacc.Bacc(
        "TRN2", target_bir_lowering=False, debug=False, num_devices=NCORES
    )
    xs_d = nc.dram_tensor("xs", [2, KP, XROWS, C], bf16, kind="ExternalInput")
    bts_d = nc.dram_tensor(
        "bts", [NG, VW, 2 * HB, RUN], bf16, kind="ExternalInput"
    )
    b4s_d = nc.dram_tensor(
        "b4s", [NG, VW, HB, RUN], bf16, kind="ExternalInput"
    )
    btw_d = nc.dram_tensor(
        "btw", [NG, 32, 2 * HB, WA], bf16, kind="ExternalInput"
    )
    btd_d = nc.dram_tensor(
        "btd", [NG, 8, 2 * HB, WB], bf16, kind="ExternalInput"
    )
    o_d = nc.dram_tensor("out", [HL, 2, 128, C], f16, kind="ExternalOutput")

    with tile.TileContext(nc) as tc:
        with (
            tc.tile_pool(name="xp", bufs=1) as xp,
            tc.tile_pool(name="bp", bufs=1) as bp,
            tc.tile_pool(name="pp", bufs=4, space="PSUM") as pp,
            tc.tile_pool(name="op", bufs=3) as op,
        ):
            # Rotating band buffers. BT is flat [KP, PADB + 16*SEG]: a 32-elem
            # scratch head per partition absorbs the base-32 scatter's dummy
            # rows (DMA partition bases must be 0/32/64), then 16 (kseg, hh)
            # segments of 320 in (wl', p, q, w0) element order.
            bts = [
                bp.tile([KP, PADB + 2 * HB * SEG], bf16, tag=f"bt{i}", name=f"bt{i}")
                for i in range(2)
            ]
            b4s = [
                bp.tile([VW, HB, MWL, 2, 2, 2], bf16, tag=f"b4{i}", name=f"b4{i}")
                for i in range(2)
            ]

            # PE warm-up: dummy matmuls burn the p-state ramp while input
            # DMAs stream, so real matmuls run at full clock from the start.
            warm = opts.get("warm", 14)
            if warm:
                wt = xp.tile([32, 288], bf16, tag="warm")
                nc.vector.memset(wt[:].bitcast(f32), 0.0)
                wps = pp.tile([32, C], f32, tag="warmps")
                for _ in range(warm):
                    nc.tensor.matmul(wps[:], wt[:, :32], wt[:, 32:288])

            # Band zeros: established once per buffer by engine memsets, in
            # the shadow of the x/band input DMAs. All groups ship sparse.
            for i in range(2):
                nc.gpsimd.memset(bts[i][:].bitcast(f32), 0.0)
                nc.vector.memset(b4s[i][:].bitcast(f32), 0.0)

            # x: interleaved w0 chunks so rows stream in compute order.
            xa, xb = [], []
            for w0 in range(2):
                xa.append(xp.tile([KP, XH, C], bf16, tag=f"xa{w0}", name=f"xa{w0}"))
                xb.append(xp.tile([KP, XROWS - XH, C], bf16, tag=f"xb{w0}", name=f"xb{w0}"))
            for w0 in range(2):
                nc.sync.dma_start(xa[w0][:], xs_d.ap()[w0, :, :XH])
            XM = 14  # mid-chunk rows
            for w0 in range(2):
                nc.sync.dma_start(
                    xb[w0][:, :XM, :], xs_d.ap()[w0, :, XH : XH + XM]
                )
            for w0 in range(2):
                nc.sync.dma_start(
                    xb[w0][:, XM:, :], xs_d.ap()[w0, :, XH + XM :]
                )

            def xrow(w0, r, parts):
                t = xa[w0] if r < XH else xb[w0]
                return t[0:parts, r if r < XH else r - XH, :]

            pitch4 = HB * SEG

            def bt_lhsT(bt, kseg, hh, w0):
                ap = bt[:]
                ap.ap = bass_rust.VecI64Pair([(PITCH, KP), (2, 128)])
                ap.offset = (
                    ap.offset
                    + PADB
                    + (kseg * HB + hh) * SEG
                    + 8 * (K - 1)
                    + w0
                )
                return ap

            def bt_scatter(bt, p0, nrows, voff):
                # Staircase scatter into partitions [p0, p0+nrows): partition
                # p0+i holds v = p0+i-VW*kb... encoded via voff = 8*v(i=0).
                dst = bt[:]
                dst.ap = bass_rust.VecI64Pair(
                    [(PITCH + 8, nrows), (SEG, 2 * HB), (1, RUN)]
                )
                dst.offset = dst.offset + p0 * PITCH + PADB + voff
                return dst

            for g in range(NG):
                bt, b4 = bts[g % 2], b4s[g % 2]
                # Sparse refresh: overwrite only the fused runs. kb0 scatters
                # the staircase from partition 0; kb1 (partitions 36..72)
                # ships as windowed-dense rectangles from the aligned bases
                # 32/64 (staircase scatters are illegal off partition 0). The
                # base-32 piece's 4 leading rows only write zeros over zeros.
                nc.scalar.dma_start(bt_scatter(bt, 0, VW, 0), bts_d.ap()[g])
                dstw = bt[:]
                dstw.ap = bass_rust.VecI64Pair(
                    [(PITCH, 32), (SEG, 2 * HB), (1, WA)]
                )
                dstw.offset = dstw.offset + 32 * PITCH + PADB
                nc.scalar.dma_start(dstw, btw_d.ap()[g])
                dstd = bt[:]
                dstd.ap = bass_rust.VecI64Pair(
                    [(PITCH, 8), (SEG, 2 * HB), (1, WB)]
                )
                dstd.offset = dstd.offset + 64 * PITCH + PADB + 8 * 28
                nc.scalar.dma_start(dstd, btd_d.ap()[g])
                dst4 = b4[:]
                dst4.ap = bass_rust.VecI64Pair(
                    [(pitch4 + 8, VW), (SEG, HB), (1, RUN)]
                )
                nc.scalar.dma_start(dst4, b4s_d.ap()[g])
                ot = op.tile([128, HB, 2, C], f16)
                for hh in range(HB):
                    h = g * HB + hh
                    ps = pp.tile([128, 2, C], f32)
                    for w0 in range(2):
                        for kseg in range(2):
                            nc.tensor.matmul(
                                ps[:, w0, :],
                                bt_lhsT(bt, kseg, hh, w0),
                                xrow(w0, h + 2 * kseg, KP),
                                start=(kseg == 0),
                                stop=False,
                            )
                        nc.tensor.matmul(
                            ps[:, w0, :],
                            b4[:, hh, K - 1 : K - 1 + 32, :, :, w0],
                            xrow(w0, h + 4, VW),
                            start=False,
                            stop=True,
                        )
                    if h % 2 == 0:
                        nc.vector.tensor_copy(ot[:, hh, :, :], ps[:])
                    else:
                        nc.scalar.copy(ot[:, hh, :, :], ps[:])
                # 4-row chunks avoid head-of-line blocking on DMA_ENGINES;
                # the final group tapers so the kernel tail is short.
                chunks = (
                    [(0, 4), (4, 8)]
                    if g < NG - 1
                    else [(0, 4), (4, 6), (6, 7), (7, 8)]
                )
                for lo, hi in chunks:
                    r0 = g * HB
                    nc.sync.dma_start(
                        o_d.ap()[r0 + lo : r0 + hi].transpose((2, 0, 1, 3)),
                        ot[:, lo:hi],
                    )

    nc.compile()
    _cache[key] = nc
    return nc


def _band_runs(kern_b, h0):
    """S[ki, v, h, 40]: fused scatter runs for this core's 32 rows.

    Run element j = dwl*8 + p*4 + q*2 + w0 holds kappa for tap (ki, kj=4-dwl)
    at output (h, wl=v-kj, w0, p, q); zero where wl falls outside [0, 32).
    """
    ks = kern_b.reshape(K, K, H, R, W, R)[:, :, h0 : h0 + HL]
    V = ks.reshape(K, K, HL, R, 2, 32, R)  # [ki, kj, h, p, w0, wl, q]
    S = np.zeros((K, VW, HL, K, R, R, 2), np.float32)
    for dwl in range(K):
        kj = K - 1 - dwl
        # V[:, kj] dims [ki, h, p, w0, wl, q] -> [ki, wl, h, p, q, w0]
        S[:, kj : kj + 32, :, dwl] = np.transpose(V[:, kj], (0, 4, 1, 2, 5, 3))
    return S.reshape(K, VW, HL, RUN)


def _prep_core(xpad_t, kern, core, bf16):
    b, hh = divmod(core, 2)
    h0 = hh * HL

    # x blocks: block kb row j = padded x row h0+j+kb (zero past the end).
    base = xpad_t[b]  # [H+4, W+4, C]
    ext = np.concatenate([base, np.zeros((1,) + base.shape[1:], base.dtype)], 0)
    xs = np.empty((2, KP, XROWS, C), np.float32)
    for w0 in range(2):
        for kb in range(NKB):
            blk = ext[h0 + kb : h0 + kb + XROWS, 32 * w0 : 32 * w0 + VW]
            xs[w0, kb * VW : (kb + 1) * VW] = np.transpose(blk, (1, 0, 2))

    S = _band_runs(kern[b], h0)  # [ki, v, h, RUN]
    Sg = S.reshape(K, VW, NG, HB, RUN)

    # Sparse payloads for groups 1..NG-1: [g, kb, v, (kseg, hh), RUN].
    bts = np.empty((NG, NKB, VW, 2, HB, RUN), np.float32)
    for kseg in range(2):
        for kb in range(NKB):
            bts[:, kb, :, kseg] = np.transpose(Sg[2 * kseg + kb], (1, 0, 2, 3))
    b4s = np.transpose(Sg[K - 1], (1, 0, 2, 3))  # [g, v, hh, RUN]

    # kb0 scatters sparse runs from partition 0; kb1 ships as two
    # windowed-dense rectangles (zeros included) from bases 32 and 64.
    a = bts.reshape(NG, NKB, VW, 2 * HB, RUN)
    ssrc = np.ascontiguousarray(a[:, 0])
    btw = np.zeros((NG, 32, 2 * HB, WA), np.float32)
    for v in range(28):
        btw[:, 4 + v, :, 8 * v : 8 * v + RUN] = a[:, 1, v]
    btd = np.zeros((NG, 8, 2 * HB, WB), np.float32)
    for j in range(8):
        btd[:, j, :, 8 * j : 8 * j + RUN] = a[:, 1, 28 + j]

    return {
        "xs": xs.astype(bf16),
        "bts": ssrc.astype(bf16),
        "b4s": b4s.astype(bf16),
        "btw": btw.astype(bf16),
        "btd": btd.astype(bf16),
    }


def _assemble(results):
    out = np.empty((B, C, H * R, W * R), np.float32)
    for i in range(NCORES):
        b, hh = divmod(i, 2)
        h0 = hh * HL
        o = np.asarray(results[i]["out"], np.float32)
        o = o.reshape(HL, 2, 32, R, R, C)  # [h, w0, wl, p, q, c]
        oc = np.transpose(o, (5, 0, 3, 1, 2, 4)).reshape(C, HL * R, W * R)
        out[b, :, h0 * R : (h0 + HL) * R, :] = oc
    return out


def _in_maps(x, kern):
    import ml_dtypes

    bf16 = ml_dtypes.bfloat16
    x_pad_t = np.pad(
        np.transpose(np.asarray(x, np.float32), (0, 2, 3, 1)),
        ((0, 0), (PAD, PAD), (PAD, PAD), (0, 0)),
    )
    kern = np.asarray(kern, np.float32)
    return [_prep_core(x_pad_t, kern, i, bf16) for i in range(NCORES)]


def kernel(x, kernel, kernel_size, ratio):
    assert int(kernel_size) == K and int(ratio) == R
    x = np.asarray(x)
    assert x.shape == (B, C, H, W), x.shape
    nc = _build()
    from concourse.bass_utils import run_bass_kernel_spmd

    res = run_bass_kernel_spmd(nc, _in_maps(x, kernel), core_ids=list(range(NCORES)))
    return _assemble(res.results)


# revision 20
# speedup vs baseline: 1.3090x; 1.0584x over previous
"""CARAFE content-aware upsampling kernel for 8 Trainium2 NeuronCores.

Math: out[b,c,2h+p,2w+q] = sum_{ki,kj} x[b,c,h+ki-2,w+kj-2] * kappa[b,ki*5+kj,2h+p,2w+q]

Per low-res row h and width-half w0, the 128 outputs (wl,p,q) form a banded
matmul over the 36-wide w' window against x rows. Tap rows are fused in pairs
into the contraction dim via shift-staged x: the x tile holds two partition
blocks kb=0,1 where block kb row j = x[j+kb], so

    mm(kseg):  psum += Band_kseg[72, 128]^T @ xt[0:72, h + 2*kseg, :]

covers taps (2*kseg, 2*kseg+1), and a final K=36 matmul covers tap 4 —
3 matmuls per output tile instead of 5.

Everything streams bf16 (fp32 PSUM accumulate); output is written fp16 and
widened on host. Band matrices are structural-zero staircases; zeros are
established once (dense group-0 DMA for buffer A, memsets for buffer B) and
later groups scatter only the 40-element fused runs (both w0 halves
interleaved) with flat partition-crossing APs.

Sharding: 8 cores = batch (4) x low-res-row halves (2).
"""

import sys

import numpy as np

if "/opt/trn_rl_repo" not in sys.path:
    sys.path.insert(0, "/opt/trn_rl_repo")

B, C, H, W = 4, 256, 64, 64
K, R = 5, 2           # kernel_size, ratio
PAD = K // 2
NCORES = 8
HL = H // 2           # low-res rows per core
VW = 32 + PAD + PAD   # contraction window width (36)
NKB = 2               # stacked tap rows per paired matmul
KP = NKB * VW         # 72 partitions for paired matmuls
XROWS = VW            # x rows staged per block (36)
HB = 8                # low-res rows per band/output group
NG = HL // HB         # 4 groups
RUN = K * 8           # 40: (kj 5) x (w0 2) x (p 2) x (q 2) fused run
MWL = VW + R * R      # 40 wl' positions per segment
SEG = MWL * 8         # 320 elems per (kseg,hh) segment
XH = 8                # x head rows (pipelining split)
PADB = 32             # scratch elems at the head of each BT partition
PITCH = PADB + 2 * HB * SEG  # BT per-partition pitch (elements)
WA = 256              # kb1 base-32 windowed-dense column width
WB = 96               # kb1 base-64 windowed-dense column width

_cache = {}


def _build(**opts):
    key = tuple(sorted(opts.items())) or "nc"
    if key in _cache:
        return _cache[key]
    import bass_rust
    import concourse.tile as tile
    from concourse import bacc, mybir

    f32 = mybir.dt.float32
    f16 = mybir.dt.float16
    bf16 = mybir.dt.bfloat16

    nc = bacc.Bacc(
        "TRN2", target_bir_lowering=False, debug=False, num_devices=NCORES
    )
    xs_d = nc.dram_tensor("xs", [2, KP, XROWS, C], bf16, kind="ExternalInput")
    bts_d = nc.dram_tensor(
        "bts", [NG, VW, 2 * HB, RUN], bf16, kind="ExternalInput"
    )
    b4s_d = nc.dram_tensor(
        "b4s", [NG, VW, HB, RUN], bf16, kind="ExternalInput"
    )
    btw_d = nc.dram_tensor(
        "btw", [NG, 32, 2 * HB, WA], bf16, kind="ExternalInput"
    )
    btd_d = nc.dram_tensor(
        "btd", [NG, 8, 2 * HB, WB], bf16, kind="ExternalInput"
    )
    o_d = nc.dram_tensor("out", [HL, 2, 128, C], f16, kind="ExternalOutput")

    with tile.TileContext(nc) as tc:
        with (
            tc.tile_pool(name="xp", bufs=1) as xp,
            tc.tile_pool(name="bp", bufs=1) as bp,
            tc.tile_pool(name="pp", bufs=4, space="PSUM") as pp,
            tc.tile_pool(name="op", bufs=3) as op,
        ):
            # Rotating band buffers. BT is flat [KP, PADB + 16*SEG]: a 32-elem
            # scratch head per partition absorbs the base-32 scatter's dummy
            # rows (DMA partition bases must be 0/32/64), then 16 (kseg, hh)
            # segments of 320 in (wl', p, q, w0) element order.
            bts = [
                bp.tile([KP, PADB + 2 * HB * SEG], bf16, tag=f"bt{i}", name=f"bt{i}")
                for i in range(2)
            ]
            b4s = [
                bp.tile([VW, HB, MWL, 2, 2, 2], bf16, tag=f"b4{i}", name=f"b4{i}")
                for i in range(2)
            ]

            # PE warm-up: dummy matmuls burn the p-state ramp while input
            # DMAs stream, so real matmuls run at full clock from the start.
            warm = opts.get("warm", 36)
            if warm:
                wt = xp.tile([32, 288], bf16, tag="warm")
                nc.vector.memset(wt[:].bitcast(f32), 0.0)
                wps = pp.tile([32, C], f32, tag="warmps")
                for _ in range(warm):
                    nc.tensor.matmul(wps[:], wt[:, :32], wt[:, 32:288])

            # Band zeros: established once per buffer by engine memsets
            # (split in halves across Pool and DVE so buffer A is ready
            # quickly), in the shadow of the x head DMAs.
            HF = (PADB + 2 * HB * SEG) // 4  # f32 elems per half
            for i in range(2):
                bf = bts[i][:].bitcast(f32)
                nc.gpsimd.memset(bf[:, :HF], 0.0)
                nc.vector.memset(bf[:, HF:], 0.0)
            for i in range(2):
                nc.vector.memset(b4s[i][:].bitcast(f32), 0.0)

            # x: head rows first; the mid/tail chunks issue after group 0's
            # band pieces so they queue behind them on the DMA engines.
            xa, xb = [], []
            for w0 in range(2):
                xa.append(xp.tile([KP, XH, C], bf16, tag=f"xa{w0}", name=f"xa{w0}"))
                xb.append(xp.tile([KP, XROWS - XH, C], bf16, tag=f"xb{w0}", name=f"xb{w0}"))
            for w0 in range(2):
                nc.sync.dma_start(xa[w0][:], xs_d.ap()[w0, :, :XH])

            def xrow(w0, r, parts):
                t = xa[w0] if r < XH else xb[w0]
                return t[0:parts, r if r < XH else r - XH, :]

            pitch4 = HB * SEG

            def bt_lhsT(bt, kseg, hh, w0):
                ap = bt[:]
                ap.ap = bass_rust.VecI64Pair([(PITCH, KP), (2, 128)])
                ap.offset = (
                    ap.offset
                    + PADB
                    + (kseg * HB + hh) * SEG
                    + 8 * (K - 1)
                    + w0
                )
                return ap

            def bt_scatter(bt, p0, nrows, voff):
                # Staircase scatter into partitions [p0, p0+nrows): partition
                # p0+i holds v = p0+i-VW*kb... encoded via voff = 8*v(i=0).
                dst = bt[:]
                dst.ap = bass_rust.VecI64Pair(
                    [(PITCH + 8, nrows), (SEG, 2 * HB), (1, RUN)]
                )
                dst.offset = dst.offset + p0 * PITCH + PADB + voff
                return dst

            for g in range(NG):
                bt, b4 = bts[g % 2], b4s[g % 2]
                # Sparse refresh: overwrite only the fused runs. kb0 scatters
                # the staircase from partition 0; kb1 (partitions 36..72)
                # ships as windowed-dense rectangles from the aligned bases
                # 32/64 (staircase scatters are illegal off partition 0). The
                # base-32 piece's 4 leading rows only write zeros over zeros.
                nc.scalar.dma_start(bt_scatter(bt, 0, VW, 0), bts_d.ap()[g])
                dstw = bt[:]
                dstw.ap = bass_rust.VecI64Pair(
                    [(PITCH, 32), (SEG, 2 * HB), (1, WA)]
                )
                dstw.offset = dstw.offset + 32 * PITCH + PADB
                nc.scalar.dma_start(dstw, btw_d.ap()[g])
                dstd = bt[:]
                dstd.ap = bass_rust.VecI64Pair(
                    [(PITCH, 8), (SEG, 2 * HB), (1, WB)]
                )
                dstd.offset = dstd.offset + 64 * PITCH + PADB + 8 * 28
                nc.scalar.dma_start(dstd, btd_d.ap()[g])
                dst4 = b4[:]
                dst4.ap = bass_rust.VecI64Pair(
                    [(pitch4 + 8, VW), (SEG, HB), (1, RUN)]
                )
                nc.scalar.dma_start(dst4, b4s_d.ap()[g])
                if g == 0:
                    XM = 14  # mid-chunk rows
                    for w0 in range(2):
                        nc.scalar.dma_start(
                            xb[w0][:, :XM, :], xs_d.ap()[w0, :, XH : XH + XM]
                        )
                    for w0 in range(2):
                        nc.scalar.dma_start(
                            xb[w0][:, XM:, :], xs_d.ap()[w0, :, XH + XM :]
                        )
                ot = op.tile([128, HB, 2, C], f16)
                for hh in range(HB):
                    h = g * HB + hh
                    ps = pp.tile([128, 2, C], f32)
                    for w0 in range(2):
                        for kseg in range(2):
                            nc.tensor.matmul(
                                ps[:, w0, :],
                                bt_lhsT(bt, kseg, hh, w0),
                                xrow(w0, h + 2 * kseg, KP),
                                start=(kseg == 0),
                                stop=False,
                            )
                        nc.tensor.matmul(
                            ps[:, w0, :],
                            b4[:, hh, K - 1 : K - 1 + 32, :, :, w0],
                            xrow(w0, h + 4, VW),
                            start=False,
                            stop=True,
                        )
                    if h % 2 == 0:
                        nc.vector.tensor_copy(ot[:, hh, :, :], ps[:])
                    else:
                        nc.scalar.copy(ot[:, hh, :, :], ps[:])
                # 4-row chunks avoid head-of-line blocking on DMA_ENGINES;
                # the final group tapers so the kernel tail is short.
                chunks = (
                    [(0, 4), (4, 8)]
                    if g < NG - 1
                    else [(0, 4), (4, 6), (6, 7), (7, 8)]
                )
                for lo, hi in chunks:
                    r0 = g * HB
                    nc.sync.dma_start(
                        o_d.ap()[r0 + lo : r0 + hi].transpose((2, 0, 1, 3)),
                        ot[:, lo:hi],
                    )

    nc.compile()
    _cache[key] = nc
    return nc


def _band_runs(kern_b, h0):
    """S[ki, v, h, 40]: fused scatter runs for this core's 32 rows.

    Run element j = dwl*8 + p*4 + q*2 + w0 holds kappa for tap (ki, kj=4-dwl)
    at output (h, wl=v-kj, w0, p, q); zero where wl falls outside [0, 32).
    """
    ks = kern_b.reshape(K, K, H, R, W, R)[:, :, h0 : h0 + HL]
    V = ks.reshape(K, K, HL, R, 2, 32, R)  # [ki, kj, h, p, w0, wl, q]
    S = np.zeros((K, VW, HL, K, R, R, 2), np.float32)
    for dwl in range(K):
        kj = K - 1 - dwl
        # V[:, kj] dims [ki, h, p, w0, wl, q] -> [ki, wl, h, p, q, w0]
        S[:, kj : kj + 32, :, dwl] = np.transpose(V[:, kj], (0, 4, 1, 2, 5, 3))
    return S.reshape(K, VW, HL, RUN)


def _prep_core(xpad_t, kern, core, bf16):
    b, hh = divmod(core, 2)
    h0 = hh * HL

    # x blocks: block kb row j = padded x row h0+j+kb (zero past the end).
    base = xpad_t[b]  # [H+4, W+4, C]
    ext = np.concatenate([base, np.zeros((1,) + base.shape[1:], base.dtype)], 0)
    xs = np.empty((2, KP, XROWS, C), np.float32)
    for w0 in range(2):
        for kb in range(NKB):
            blk = ext[h0 + kb : h0 + kb + XROWS, 32 * w0 : 32 * w0 + VW]
            xs[w0, kb * VW : (kb + 1) * VW] = np.transpose(blk, (1, 0, 2))

    S = _band_runs(kern[b], h0)  # [ki, v, h, RUN]
    Sg = S.reshape(K, VW, NG, HB, RUN)

    # Sparse payloads for groups 1..NG-1: [g, kb, v, (kseg, hh), RUN].
    bts = np.empty((NG, NKB, VW, 2, HB, RUN), np.float32)
    for kseg in range(2):
        for kb in range(NKB):
            bts[:, kb, :, kseg] = np.transpose(Sg[2 * kseg + kb], (1, 0, 2, 3))
    b4s = np.transpose(Sg[K - 1], (1, 0, 2, 3))  # [g, v, hh, RUN]

    # kb0 scatters sparse runs from partition 0; kb1 ships as two
    # windowed-dense rectangles (zeros included) from bases 32 and 64.
    a = bts.reshape(NG, NKB, VW, 2 * HB, RUN)
    ssrc = np.ascontiguousarray(a[:, 0])
    btw = np.zeros((NG, 32, 2 * HB, WA), np.float32)
    for v in range(28):
        btw[:, 4 + v, :, 8 * v : 8 * v + RUN] = a[:, 1, v]
    btd = np.zeros((NG, 8, 2 * HB, WB), np.float32)
    for j in range(8):
        btd[:, j, :, 8 * j : 8 * j + RUN] = a[:, 1, 28 + j]

    return {
        "xs": xs.astype(bf16),
        "bts": ssrc.astype(bf16),
        "b4s": b4s.astype(bf16),
        "btw": btw.astype(bf16),
        "btd": btd.astype(bf16),
    }


def _assemble(results):
    out = np.empty((B, C, H * R, W * R), np.float32)
    for i in range(NCORES):
        b, hh = divmod(i, 2)
        h0 = hh * HL
        o = np.asarray(results[i]["out"], np.float32)
        o = o.reshape(HL, 2, 32, R, R, C)  # [h, w0, wl, p, q, c]
        oc = np.transpose(o, (5, 0, 3, 1, 2, 4)).reshape(C, HL * R, W * R)
        out[b, :, h0 * R : (h0 + HL) * R, :] = oc
    return out


def _in_maps(x, kern):
    import ml_dtypes

    bf16 = ml_dtypes.bfloat16
    x_pad_t = np.pad(
        np.transpose(np.asarray(x, np.float32), (0, 2, 3, 1)),
        ((0, 0), (PAD, PAD), (PAD, PAD), (0, 0)),
    )
    kern = np.asarray(kern, np.float32)
    return [_prep_core(x_pad_t, kern, i, bf16) for i in range(NCORES)]


def kernel(x, kernel, kernel_size, ratio):
    assert int(kernel_size) == K and int(ratio) == R
    x = np.asarray(x)
    assert x.shape == (B, C, H, W), x.shape
    nc = _build()
    from concourse.bass_utils import run_bass_kernel_spmd

    res = run_bass_kernel_spmd(nc, _in_maps(x, kernel), core_ids=list(range(NCORES)))
    return _assemble(res.results)
